# revision 8
# baseline (speedup 1.0000x reference)
"""Trainium2 Bass kernel for ContextQueryAttention (trilinear similarity +
row/col softmax attention).

Full-input contract: kernel(**inputs) takes the complete arrays
  q  [16, 128, 512]   f32
  c  [16, 128, 4096]  f32
  w1 [1, 128] w2 [1, 128] w3 [1, 128] f32
and returns out [16, 512, 4096] f32 = concat([c, a, c*a, c*b], axis=1).

Sharding: data-parallel over batch B=16 across 8 NeuronCores (2 batches per
core), no collectives.

Structure: streaming n-windows of 1024 across both batches (8 global
windows), software-pipelined so PE / ACT / DVE / GPSIMD / DMA overlap:

  per window w (8 n-chunks):
    pass2:  s' = A^T q per chunk -> ACT exp -> expS [n,m] (unscaled, missing
            e^{out2[n]})
    scale:  DVE tensor_scalar expS *= e2[n] in place, accum_out -> true
            rowsum (fused, 4x mode)
    cE:     PE chunk transposes of c (plain c^T, bf16)
    tmp:    accumulates cE^T @ expSE over all windows (PSUM resident)
    colsum: ones^T @ expSE accumulated over all windows ([1,512] PSUM)
    pass1:  s^T = Bq^T c + out1[m] bias -> ACT exp -> expST [m,n] (true exp)
    a-wave (lagged one window): qT^T @ expST, normalized by rowinv via DVE,
            ca = a*c; stores stream out per window
  batch b's b-waves (tmpT^T @ expST, cb = c*b) are interleaved into batch
  b+1's windows / the tail.

  rowsum/rowinv: e2 cancels for the row softmax; rowinv = 1/(rowsum'*e2)
  computed per window, PE-transposed to a [1,1024] row and
  gpsimd-partition-broadcast to [128,1024] for the normalization TTs.
  colsum -> colinv via DMA rearrange [1,512]->[128,4] + reciprocal; applied
  to tmpT (per-partition scalar).

All matmul operands bf16; exp outputs written bf16 by ACT directly.
"""

import sys

import numpy as np

try:
    import concourse.bass as bass  # noqa: F401
except Exception:  # pragma: no cover
    sys.path.insert(0, "/opt/trn_rl_repo")
    import concourse.bass as bass  # noqa: F401

import concourse.bacc as bacc
import concourse.mybir as mybir
import concourse.tile as tile
from concourse.masks import make_identity

F32 = mybir.dt.float32
BF16 = mybir.dt.bfloat16

# Problem geometry (hardcoded per contract)
B = 16          # total batches
NCORES = 8
CB = B // NCORES  # batches per core = 2
D = 128         # model dim == partition count
M = 512         # query length
N = 4096        # context length
P = 128
MCH = M // P    # 4 m-chunks of 128
WIN = 1024      # n-window width
NWIN = N // WIN  # 4 windows per batch
CHW = WIN // P   # 8 n-chunks per window
T = CB * NWIN    # 8 global windows


def build_body(tc, q_ap, c_ap, w1_ap, w2_ap, w3_ap, out_ap):
    """Emit the per-core program. q_ap [CB,128,512], c_ap [CB,128,4096],
    w*_ap [1,128], out_ap [CB,512,4096]."""
    from contextlib import ExitStack

    nc = tc.nc
    mult = mybir.AluOpType.mult
    add = mybir.AluOpType.add
    Exp = mybir.ActivationFunctionType.Exp

    with ExitStack() as ctx:
        consts = ctx.enter_context(tc.tile_pool(name="consts", bufs=1))
        per = ctx.enter_context(tc.tile_pool(name="per", bufs=2))
        win = ctx.enter_context(tc.tile_pool(name="win", bufs=2))
        outp = ctx.enter_context(tc.tile_pool(name="outp", bufs=2))
        pp_s = ctx.enter_context(tc.tile_pool(name="pp_s", bufs=2, space="PSUM"))
        pp_ab = ctx.enter_context(tc.tile_pool(name="pp_ab", bufs=2, space="PSUM"))
        pp_acc = ctx.enter_context(tc.tile_pool(name="pp_acc", bufs=1, space="PSUM"))

        identity = consts.tile([P, P], F32)
        make_identity(nc, identity)
        identity_b = consts.tile([P, P], BF16)
        nc.vector.tensor_copy(identity_b, identity)
        ones_b = consts.tile([P, 1], BF16)
        nc.vector.memset(ones_b, 1.0)
        w1c = consts.tile([P, 1], F32)
        w2c = consts.tile([P, 1], F32)
        w3c = consts.tile([P, 1], F32)
        nc.sync.dma_start(out=w1c, in_=w1_ap.rearrange("o d -> d o"))
        nc.sync.dma_start(out=w2c, in_=w2_ap.rearrange("o d -> d o"))
        nc.sync.dma_start(out=w3c, in_=w3_ap.rearrange("o d -> d o"))
        w1b = consts.tile([P, 1], BF16)
        w2b = consts.tile([P, 1], BF16)
        nc.vector.tensor_copy(w1b, w1c)
        nc.vector.tensor_copy(w2b, w2c)

        state = [dict() for _ in range(CB)]

        def prep_load(b):
            """Acquire per-batch input tiles and issue loads."""
            st = state[b]
            st["c_t"] = per.tile([P, N], F32, tag="ct", name=f"ct{b}")
            st["q_b"] = per.tile([P, M], BF16, tag="qb", name=f"qb{b}")
            nc.gpsimd.dma_start(out=st["q_b"], in_=q_ap[b])

        def load_cwin(t):
            b, w = divmod(t, NWIN)
            lo = w * WIN
            nc.sync.dma_start(
                out=state[b]["c_t"][:, lo : lo + WIN],
                in_=c_ap[b][:, lo : lo + WIN],
            )

        def prep_compute(b):
            st = state[b]
            q_b = st["q_b"]
            # qT chunks for the a-waves
            qT = per.tile([P, M], BF16, tag="qT", name=f"qT{b}")
            for i in range(MCH):
                ps_q = pp_ab.tile([P, P], BF16, tag="ab")
                nc.tensor.transpose(ps_q, q_b[:, i * P : (i + 1) * P], identity_b)
                nc.vector.tensor_copy(qT[:, i * P : (i + 1) * P], ps_q)
            st["qT"] = qT
            # Bq = q*w3 + w2 (pass-1 stationary)
            Bq = per.tile([P, M], BF16, tag="Bq", name=f"Bq{b}")
            nc.vector.tensor_scalar(Bq, q_b, w3c, w2c, mult, add)
            st["Bq"] = Bq
            # out1[m] column layout (pass-1 bias)
            ps_o1 = pp_ab.tile([P, MCH], F32, tag="ab")
            for i in range(MCH):
                nc.tensor.matmul(
                    ps_o1[:, i : i + 1],
                    lhsT=q_b[:, i * P : (i + 1) * P],
                    rhs=w1b,
                    start=True,
                    stop=True,
                )
            o1col = per.tile([P, MCH], F32, tag="o1", name=f"o1{b}")
            nc.vector.tensor_copy(o1col, ps_o1)
            st["o1col"] = o1col
            # persistent per-batch big tensors + accumulators
            st["expST"] = per.tile([P, MCH, N], BF16, tag="expST", name=f"eST{b}")
            st["rowinvb"] = per.tile([P, N], F32, tag="rowinvb", name=f"riv{b}")
            st["tmp_ps"] = pp_acc.tile([P, M], F32, tag="tmp", name=f"tmp{b}")
            st["cols_ps"] = pp_acc.tile([1, M], F32, tag="cols", name=f"cols{b}")

        def emit_awave(t):
            b, w = divmod(t, NWIN)
            st = state[b]
            lo = w * WIN
            for h in range(2):
                l2 = lo + h * M
                ps_a = pp_ab.tile([P, M], F32, tag="ab")
                for i in range(MCH):
                    nc.tensor.matmul(
                        ps_a,
                        lhsT=st["qT"][:, i * P : (i + 1) * P],
                        rhs=st["expST"][:, i, l2 : l2 + M],
                        start=(i == 0),
                        stop=(i == MCH - 1),
                    )
                a_t = outp.tile([P, M], F32, tag="a")
                nc.vector.tensor_tensor(a_t, ps_a, st["rowinvb"][:, l2 : l2 + M], mult)
                nc.gpsimd.dma_start(out=out_ap[b, P : 2 * P, l2 : l2 + M], in_=a_t)
                ca_t = outp.tile([P, M], F32, tag="ca")
                nc.vector.tensor_tensor(ca_t, a_t, st["c_t"][:, l2 : l2 + M], mult)
                nc.gpsimd.dma_start(
                    out=out_ap[b, 2 * P : 3 * P, l2 : l2 + M], in_=ca_t
                )

        def emit_bprep(b):
            st = state[b]
            tmpU = per.tile([P, M], BF16, tag="tmpU", name=f"tmpU{b}")
            nc.vector.tensor_copy(tmpU, st["tmp_ps"])
            colsRow = per.tile([1, M], F32, tag="colsRow", name=f"csR{b}")
            nc.vector.tensor_copy(colsRow, st["cols_ps"])
            colC = per.tile([P, MCH], F32, tag="colC", name=f"colC{b}")
            for i in range(MCH):
                nc.sync.dma_start(
                    out=colC[:, i : i + 1],
                    in_=colsRow[0:1, i * P : (i + 1) * P],
                )
            colinv = per.tile([P, MCH], F32, tag="colinv", name=f"cinv{b}")
            nc.vector.reciprocal(colinv, colC)
            tmpT = per.tile([P, M], BF16, tag="tmpT", name=f"tmpT{b}")
            for i in range(MCH):
                ps_tt = pp_ab.tile([P, P], BF16, tag="ab")
                nc.tensor.transpose(ps_tt, tmpU[:, i * P : (i + 1) * P], identity_b)
                nc.vector.tensor_scalar(
                    tmpT[:, i * P : (i + 1) * P],
                    ps_tt,
                    colinv[:, i : i + 1],
                    None,
                    mult,
                )
            st["tmpT"] = tmpT

        def emit_bwave(b, w):
            st = state[b]
            lo = w * WIN
            for h in range(2):
                l2 = lo + h * M
                ps_b = pp_ab.tile([P, M], F32, tag="ab")
                for i in range(MCH):
                    nc.tensor.matmul(
                        ps_b,
                        lhsT=st["tmpT"][:, i * P : (i + 1) * P],
                        rhs=st["expST"][:, i, l2 : l2 + M],
                        start=(i == 0),
                        stop=(i == MCH - 1),
                    )
                b1_t = outp.tile([P, M], F32, tag="b1")
                nc.vector.tensor_tensor(
                    b1_t, ps_b, st["rowinvb"][:, l2 : l2 + M], mult
                )
                cb_t = outp.tile([P, M], F32, tag="cb")
                nc.gpsimd.tensor_tensor(cb_t, b1_t, st["c_t"][:, l2 : l2 + M], mult)
                nc.sync.dma_start(
                    out=out_ap[b, 3 * P : 4 * P, l2 : l2 + M], in_=cb_t
                )

        def emit_window(t):
            b, w = divmod(t, NWIN)
            st = state[b]
            lo = w * WIN

            # prefetch c window t+2; load batch-1 q just ahead of its windows
            if t + 2 < T:
                b2 = (t + 2) // NWIN
                if (t + 2) % NWIN == 0:
                    prep_load(b2)
                load_cwin(t + 2)
            # out block 0 is just c: DRAM->DRAM, no SBUF involved
            nc.sync.dma_start(
                out=out_ap[b, 0:P, lo : lo + WIN], in_=c_ap[b][:, lo : lo + WIN]
            )

            # a-wave of the previous global window (deps all resolved)
            if t > 0:
                emit_awave(t - 1)

            if w == 0:
                prep_compute(b)
                if b == 1:
                    # batch-0 accumulators are complete; drain them now so
                    # batch-1's tmp/cols PSUM slots free up early
                    emit_bprep(0)

            # c window -> bf16 + A = c*w3 + w1
            c_b_w = win.tile([P, WIN], BF16, tag="cbw")
            nc.vector.tensor_copy(c_b_w, st["c_t"][:, lo : lo + WIN])
            A_w = win.tile([P, WIN], BF16, tag="A")
            nc.vector.tensor_scalar(A_w, c_b_w, w3c, w1c, mult, add)

            # out2 chunk columns -> e2 = exp(out2)
            ps_o2 = pp_ab.tile([P, CHW], F32, tag="ab")
            for j in range(CHW):
                nc.tensor.matmul(
                    ps_o2[:, j : j + 1],
                    lhsT=c_b_w[:, j * P : (j + 1) * P],
                    rhs=w2b,
                    start=True,
                    stop=True,
                )
            e2col_w = win.tile([P, CHW], F32, tag="e2")
            nc.scalar.activation(e2col_w, ps_o2, Exp, bias=0.0, scale=1.0)

            # pass 2: expS' = exp(s - out2) per chunk, [n,m] layout
            expS_w = win.tile([P, CHW, M], BF16, tag="expS")
            for jj in range(CHW // 2):
                ps2 = pp_s.tile([P, 2 * M], F32, tag="s")
                for h in range(2):
                    j = 2 * jj + h
                    nc.tensor.matmul(
                        ps2[:, h * M : (h + 1) * M],
                        lhsT=A_w[:, j * P : (j + 1) * P],
                        rhs=st["q_b"],
                        start=True,
                        stop=True,
                    )
                nc.scalar.activation(
                    expS_w[:, 2 * jj : 2 * jj + 2, :], ps2, Exp, bias=0.0, scale=1.0
                )

            # scale to true exp in place; fused accum gives true rowsum
            rowsumC_w = win.tile([P, CHW], F32, tag="rs")
            for j in range(CHW):
                nc.vector.tensor_scalar(
                    expS_w[:, j, :],
                    expS_w[:, j, :],
                    e2col_w[:, j : j + 1],
                    None,
                    mult,
                    add,
                    accum_out=rowsumC_w[:, j : j + 1],
                )
            rowinvC_w = win.tile([P, CHW], F32, tag="ri")
            nc.vector.reciprocal(rowinvC_w, rowsumC_w)

            # cE = plain c^T chunks (e2 already folded into expSE)
            cE_w = win.tile([P, CHW, P], BF16, tag="cE")
            for x in range(CHW // 4):
                ps_ct = pp_ab.tile([P, 4, P], BF16, tag="ab")
                for k in range(4):
                    j = 4 * x + k
                    nc.tensor.transpose(
                        ps_ct[:, k, :], c_b_w[:, j * P : (j + 1) * P], identity_b
                    )
                nc.vector.tensor_copy(cE_w[:, 4 * x : 4 * x + 4, :], ps_ct)

            # tmp and colsum accumulate across all windows of the batch
            for j in range(CHW):
                nc.tensor.matmul(
                    st["tmp_ps"],
                    lhsT=cE_w[:, j, :],
                    rhs=expS_w[:, j, :],
                    start=(w == 0 and j == 0),
                    stop=(w == NWIN - 1 and j == CHW - 1),
                )
            for j in range(CHW):
                nc.tensor.matmul(
                    st["cols_ps"],
                    lhsT=ones_b,
                    rhs=expS_w[:, j, :],
                    start=(w == 0 and j == 0),
                    stop=(w == NWIN - 1 and j == CHW - 1),
                )

            # pass 1: true exp(s) in [m,n] layout (out1 bias per m-chunk)
            for i in range(MCH):
                ps1 = pp_s.tile([P, WIN], F32, tag="s")
                for h in range(2):
                    nc.tensor.matmul(
                        ps1[:, h * M : (h + 1) * M],
                        lhsT=st["Bq"][:, i * P : (i + 1) * P],
                        rhs=c_b_w[:, h * M : (h + 1) * M],
                        start=True,
                        stop=True,
                    )
                nc.scalar.activation(
                    st["expST"][:, i, lo : lo + WIN],
                    ps1,
                    Exp,
                    bias=st["o1col"][:, i : i + 1],
                    scale=1.0,
                )

            # b-phase of the previous batch rides along in this batch's windows
            if b == 1:
                emit_bwave(0, w)

            # rowinv -> [1,WIN] row -> broadcast (consumed by next window's
            # a-wave; emitted last so it never head-of-line blocks PE)
            ps_rT = pp_ab.tile([CHW, P], F32, tag="ab")
            nc.tensor.transpose(ps_rT, rowinvC_w, identity)
            rowT_w = win.tile([CHW, P], F32, tag="rT")
            nc.vector.tensor_copy(rowT_w, ps_rT)
            rowrow_w = win.tile([1, WIN], F32, tag="rr")
            nc.sync.dma_start(
                out=rowrow_w.rearrange("p (a b) -> p a b", a=CHW), in_=rowT_w
            )
            nc.gpsimd.partition_broadcast(st["rowinvb"][:, lo : lo + WIN], rowrow_w)

        # ---- fill ----
        prep_load(0)
        load_cwin(0)
        load_cwin(1)
        for t in range(T):
            emit_window(t)
        # ---- drain ----
        emit_awave(T - 1)
        emit_bprep(1)
        for w in range(NWIN):
            emit_bwave(1, w)


_PROGRAM = None


def _build_program(loops=None):
    """Build the per-core Bass program. loops=None -> straight-line (grading
    path); loops=R -> wrap the body in a Tile For_i repetition loop (used
    only for steady-state benchmarking)."""
    nc = bacc.Bacc("TRN2", target_bir_lowering=False, debug=False, num_devices=NCORES)
    q_d = nc.dram_tensor("q", [CB, D, M], F32, kind="ExternalInput")
    c_d = nc.dram_tensor("c", [CB, D, N], F32, kind="ExternalInput")
    w1_d = nc.dram_tensor("w1", [1, D], F32, kind="ExternalInput")
    w2_d = nc.dram_tensor("w2", [1, D], F32, kind="ExternalInput")
    w3_d = nc.dram_tensor("w3", [1, D], F32, kind="ExternalInput")
    out_d = nc.dram_tensor("out", [CB, 4 * D, N], F32, kind="ExternalOutput")
    with tile.TileContext(nc) as tc:
        if loops is None:
            build_body(
                tc, q_d.ap(), c_d.ap(), w1_d.ap(), w2_d.ap(), w3_d.ap(), out_d.ap()
            )
        else:
            with tc.For_i(0, loops, 1):
                build_body(
                    tc,
                    q_d.ap(),
                    c_d.ap(),
                    w1_d.ap(),
                    w2_d.ap(),
                    w3_d.ap(),
                    out_d.ap(),
                )
    nc.compile()
    return nc


def _get_program():
    global _PROGRAM
    if _PROGRAM is None:
        _PROGRAM = _build_program()
    return _PROGRAM


def kernel(q, c, w1, w2, w3, _collect_results=None):
    q = np.ascontiguousarray(q, dtype=np.float32)
    c = np.ascontiguousarray(c, dtype=np.float32)
    w1 = np.ascontiguousarray(w1, dtype=np.float32)
    w2 = np.ascontiguousarray(w2, dtype=np.float32)
    w3 = np.ascontiguousarray(w3, dtype=np.float32)

    nc = _get_program()
    in_maps = [
        {
            "q": q[CB * i : CB * (i + 1)],
            "c": c[CB * i : CB * (i + 1)],
            "w1": w1,
            "w2": w2,
            "w3": w3,
        }
        for i in range(NCORES)
    ]
    from concourse import bass_utils

    res = bass_utils.run_bass_kernel_spmd(nc, in_maps, core_ids=list(range(NCORES)))
    if _collect_results is not None:
        _collect_results.append(res)
    return np.concatenate([r["out"] for r in res.results], axis=0)


# revision 12
# speedup vs baseline: 1.1031x; 1.1031x over previous
"""Trainium2 Bass kernel for ContextQueryAttention (trilinear similarity +
row/col softmax attention).

Full-input contract: kernel(**inputs) takes the complete arrays
  q  [16, 128, 512]   f32
  c  [16, 128, 4096]  f32
  w1 [1, 128] w2 [1, 128] w3 [1, 128] f32
and returns out [16, 512, 4096] f32 = concat([c, a, c*a, c*b], axis=1).

Sharding: data-parallel over batch B=16 across 8 NeuronCores (2 batches per
core), no collectives.

Structure: streaming n-windows of 1024 across both batches (8 global
windows), software-pipelined so PE / ACT / DVE / GPSIMD / DMA overlap:

  per window w (8 n-chunks of 128):
    pass2:  s' = A^T q per chunk -> ACT exp -> expS [n,m] (unscaled; the
            e^{out2[n]} factor cancels in the row softmax)
    rowsum: one DVE reduce per window (off the PE critical path), then
            rowinv = 1/(rowsum*e2) -> PE transpose -> [1,1024] row ->
            gpsimd partition_broadcast
    cE:     PE chunk transposes of c, drained with a per-chunk
            tensor_scalar that folds e2[n] in (cE = c^T * e2)
    tmp:    accumulates cE^T @ expS over all windows (PSUM resident)
    pass1:  s^T = Bq^T c + out1[m] bias -> ACT exp (true exp, FD1024) with
            accum_out -> colsum partials
    a-wave: qT^T @ expST at end of the window (FD1024); its rowinv
            normalization + ca = a*c (gpsimd) land at the head of the next
            window so they never head-of-line block anything
  batch b's b-waves (tmpT^T @ expST, cb = c*b) ride along in batch b+1's
  windows / the tail.

All matmul operands bf16; exp outputs written bf16 by ACT directly; out
block 0 (the c passthrough) is DRAM->DRAM DMA, untouched by compute.
"""

import sys

import numpy as np

try:
    import concourse.bass as bass  # noqa: F401
except Exception:  # pragma: no cover
    sys.path.insert(0, "/opt/trn_rl_repo")
    import concourse.bass as bass  # noqa: F401

import concourse.bacc as bacc
import concourse.mybir as mybir
import concourse.tile as tile
from concourse.masks import make_identity

F32 = mybir.dt.float32
BF16 = mybir.dt.bfloat16

# Problem geometry (hardcoded per contract)
B = 16          # total batches
NCORES = 8
CB = B // NCORES  # batches per core = 2
D = 128         # model dim == partition count
M = 512         # query length
N = 4096        # context length
P = 128
MCH = M // P    # 4 m-chunks of 128
WIN = 1024      # n-window width
NWIN = N // WIN  # 4 windows per batch
CHW = WIN // P   # 8 n-chunks per window
T = CB * NWIN    # 8 global windows


def build_body(tc, q_ap, c_ap, w1_ap, w2_ap, w3_ap, out_ap):
    """Emit the per-core program. q_ap [CB,128,512], c_ap [CB,128,4096],
    w*_ap [1,128], out_ap [CB,512,4096]."""
    from contextlib import ExitStack

    nc = tc.nc
    mult = mybir.AluOpType.mult
    add = mybir.AluOpType.add
    Exp = mybir.ActivationFunctionType.Exp
    AxX = mybir.AxisListType.X

    with ExitStack() as ctx:
        consts = ctx.enter_context(tc.tile_pool(name="consts", bufs=1))
        per = ctx.enter_context(tc.tile_pool(name="per", bufs=2))
        win = ctx.enter_context(tc.tile_pool(name="win", bufs=2))
        outp = ctx.enter_context(tc.tile_pool(name="outp", bufs=2))
        pp_s = ctx.enter_context(tc.tile_pool(name="pp_s", bufs=2, space="PSUM"))
        pp_ab = ctx.enter_context(tc.tile_pool(name="pp_ab", bufs=3, space="PSUM"))
        pp_acc = ctx.enter_context(tc.tile_pool(name="pp_acc", bufs=1, space="PSUM"))

        identity = consts.tile([P, P], F32)
        make_identity(nc, identity)
        identity_b = consts.tile([P, P], BF16)
        nc.vector.tensor_copy(identity_b, identity)
        w1c = consts.tile([P, 1], F32)
        w2c = consts.tile([P, 1], F32)
        w3c = consts.tile([P, 1], F32)
        nc.sync.dma_start(out=w1c, in_=w1_ap.rearrange("o d -> d o"))
        nc.sync.dma_start(out=w2c, in_=w2_ap.rearrange("o d -> d o"))
        nc.sync.dma_start(out=w3c, in_=w3_ap.rearrange("o d -> d o"))
        w1b = consts.tile([P, 1], BF16)
        w2b = consts.tile([P, 1], BF16)
        nc.vector.tensor_copy(w1b, w1c)
        nc.vector.tensor_copy(w2b, w2c)

        state = [dict() for _ in range(CB)]
        pend_a = []
        pend_b = []

        def prep_load(b):
            """Acquire per-batch input tiles and issue loads."""
            st = state[b]
            st["c_t"] = per.tile([P, N], F32, tag="ct", name=f"ct{b}")
            st["q_b"] = per.tile([P, M], BF16, tag="qb", name=f"qb{b}")
            nc.gpsimd.dma_start(out=st["q_b"], in_=q_ap[b])

        def load_cwin(t):
            b, w = divmod(t, NWIN)
            lo = w * WIN
            nc.sync.dma_start(
                out=state[b]["c_t"][:, lo : lo + WIN],
                in_=c_ap[b][:, lo : lo + WIN],
            )

        def prep_compute(b):
            st = state[b]
            q_b = st["q_b"]
            # qT chunks for the a-waves
            qT = per.tile([P, M], BF16, tag="qT", name=f"qT{b}")
            for i in range(MCH):
                ps_q = pp_ab.tile([P, P], BF16, tag="ab")
                nc.tensor.transpose(ps_q, q_b[:, i * P : (i + 1) * P], identity_b)
                nc.vector.tensor_copy(qT[:, i * P : (i + 1) * P], ps_q)
            st["qT"] = qT
            # Bq = q*w3 + w2 (pass-1 stationary)
            Bq = per.tile([P, M], BF16, tag="Bq", name=f"Bq{b}")
            nc.vector.tensor_scalar(Bq, q_b, w3c, w2c, mult, add)
            st["Bq"] = Bq
            # out1[m] column layout (pass-1 bias)
            ps_o1 = pp_ab.tile([P, MCH], F32, tag="ab")
            for i in range(MCH):
                nc.tensor.matmul(
                    ps_o1[:, i : i + 1],
                    lhsT=q_b[:, i * P : (i + 1) * P],
                    rhs=w1b,
                    start=True,
                    stop=True,
                )
            o1col = per.tile([P, MCH], F32, tag="o1", name=f"o1{b}")
            nc.vector.tensor_copy(o1col, ps_o1)
            st["o1col"] = o1col
            # persistent per-batch big tensors + accumulators
            st["expST"] = per.tile([P, MCH, N], BF16, tag="expST", name=f"eST{b}")
            st["rowinvb"] = per.tile([P, N], BF16, tag="rowinvb", name=f"riv{b}")
            st["tmp_ps"] = pp_acc.tile([P, M], F32, tag="tmp", name=f"tmp{b}")
            st["colsumU"] = per.tile([P, MCH, NWIN], F32, tag="csU", name=f"csU{b}")

        def awave_mm(t):
            """a-wave matmuls at the end of window t (expST just written)."""
            b, w = divmod(t, NWIN)
            st = state[b]
            lo = w * WIN
            ps_a = pp_s.tile([P, WIN], F32, tag="s")
            for i in range(MCH):
                for h in range(2):
                    nc.tensor.matmul(
                        ps_a[:, h * M : (h + 1) * M],
                        lhsT=st["qT"][:, i * P : (i + 1) * P],
                        rhs=st["expST"][:, i, lo + h * M : lo + (h + 1) * M],
                        start=(i == 0),
                        stop=(i == MCH - 1),
                    )
            pend_a.append((ps_a, b, lo))

        def awave_finish():
            """Normalize + produce ca + store, at the head of the next window."""
            while pend_a:
                ps_a, b, lo = pend_a.pop(0)
                st = state[b]
                a_t = outp.tile([P, WIN], F32, tag="a")
                nc.vector.tensor_tensor(
                    a_t, ps_a, st["rowinvb"][:, lo : lo + WIN], mult
                )
                nc.sync.dma_start(out=out_ap[b, P : 2 * P, lo : lo + WIN], in_=a_t)
                ca_t = outp.tile([P, WIN], F32, tag="ca")
                nc.gpsimd.tensor_tensor(ca_t, a_t, st["c_t"][:, lo : lo + WIN], mult)
                nc.sync.dma_start(
                    out=out_ap[b, 2 * P : 3 * P, lo : lo + WIN], in_=ca_t
                )

        def emit_bprep(b):
            st = state[b]
            tmpU = per.tile([P, M], BF16, tag="tmpU", name=f"tmpU{b}")
            nc.vector.tensor_copy(tmpU, st["tmp_ps"])
            colsum = per.tile([P, MCH], F32, tag="cs", name=f"cs{b}")
            nc.vector.reduce_sum(colsum, st["colsumU"], axis=AxX)
            colinv = per.tile([P, MCH], F32, tag="colinv", name=f"cinv{b}")
            nc.vector.reciprocal(colinv, colsum)
            tmpT = per.tile([P, M], BF16, tag="tmpT", name=f"tmpT{b}")
            for i in range(MCH):
                ps_tt = pp_ab.tile([P, P], BF16, tag="ab")
                nc.tensor.transpose(ps_tt, tmpU[:, i * P : (i + 1) * P], identity_b)
                nc.vector.tensor_scalar(
                    tmpT[:, i * P : (i + 1) * P],
                    ps_tt,
                    colinv[:, i : i + 1],
                    None,
                    mult,
                )
            st["tmpT"] = tmpT

        def bwave_mm(b, w):
            st = state[b]
            lo = w * WIN
            ps_b = pp_s.tile([P, WIN], F32, tag="s")
            for i in range(MCH):
                for h in range(2):
                    nc.tensor.matmul(
                        ps_b[:, h * M : (h + 1) * M],
                        lhsT=st["tmpT"][:, i * P : (i + 1) * P],
                        rhs=st["expST"][:, i, lo + h * M : lo + (h + 1) * M],
                        start=(i == 0),
                        stop=(i == MCH - 1),
                    )
            pend_b.append((ps_b, b, lo))

        def bwave_finish():
            while pend_b:
                ps_b, b, lo = pend_b.pop(0)
                st = state[b]
                b1_t = outp.tile([P, WIN], F32, tag="b1")
                nc.vector.tensor_tensor(
                    b1_t, ps_b, st["rowinvb"][:, lo : lo + WIN], mult
                )
                cb_t = outp.tile([P, WIN], F32, tag="cb")
                nc.gpsimd.tensor_tensor(cb_t, b1_t, st["c_t"][:, lo : lo + WIN], mult)
                nc.sync.dma_start(
                    out=out_ap[b, 3 * P : 4 * P, lo : lo + WIN], in_=cb_t
                )

        def emit_window(t):
            b, w = divmod(t, NWIN)
            st = state[b]
            lo = w * WIN

            # prefetch c window t+2; batch-1 inputs load just ahead
            if t + 2 < T:
                if (t + 2) % NWIN == 0:
                    prep_load((t + 2) // NWIN)
                load_cwin(t + 2)
            # out block 0 is just c: DRAM->DRAM, no SBUF involved
            nc.sync.dma_start(
                out=out_ap[b, 0:P, lo : lo + WIN], in_=c_ap[b][:, lo : lo + WIN]
            )

            # finish the previous window's output waves first (DVE/gpsimd
            # heads; their deps resolved mid-previous-window)
            awave_finish()
            bwave_finish()

            if w == 0:
                prep_compute(b)
                if b == 1:
                    # batch-0 accumulators are complete; drain them now so
                    # batch-1's tmp PSUM slot frees up and tmpT is ready
                    # for the interleaved b-waves
                    emit_bprep(0)

            # c window -> bf16 + A = c*w3 + w1
            c_b_w = win.tile([P, WIN], BF16, tag="cbw")
            nc.vector.tensor_copy(c_b_w, st["c_t"][:, lo : lo + WIN])
            A_w = win.tile([P, WIN], BF16, tag="A")
            nc.vector.tensor_scalar(A_w, c_b_w, w3c, w1c, mult, add)

            # out2 chunk columns -> e2 = exp(out2)
            ps_o2 = pp_ab.tile([P, CHW], F32, tag="ab")
            for j in range(CHW):
                nc.tensor.matmul(
                    ps_o2[:, j : j + 1],
                    lhsT=c_b_w[:, j * P : (j + 1) * P],
                    rhs=w2b,
                    start=True,
                    stop=True,
                )
            e2col_w = win.tile([P, CHW], F32, tag="e2")
            nc.scalar.activation(e2col_w, ps_o2, Exp, bias=0.0, scale=1.0)

            # pass 2: expS' = exp(s - out2) per chunk, [n,m] layout
            expS_w = win.tile([P, CHW, M], BF16, tag="expS")
            for jj in range(CHW // 2):
                ps2 = pp_s.tile([P, 2 * M], F32, tag="s")
                for h in range(2):
                    j = 2 * jj + h
                    nc.tensor.matmul(
                        ps2[:, h * M : (h + 1) * M],
                        lhsT=A_w[:, j * P : (j + 1) * P],
                        rhs=st["q_b"],
                        start=True,
                        stop=True,
                    )
                nc.scalar.activation(
                    expS_w[:, 2 * jj : 2 * jj + 2, :], ps2, Exp, bias=0.0, scale=1.0
                )

            # rowsum (one reduce, off the PE critical path) -> rowinv
            rowsumC_w = win.tile([P, CHW], F32, tag="rs")
            nc.vector.reduce_sum(rowsumC_w, expS_w, axis=AxX)
            rowprod_w = win.tile([P, CHW], F32, tag="rp")
            nc.vector.tensor_tensor(rowprod_w, rowsumC_w, e2col_w, mult)
            rowinvC_w = win.tile([P, CHW], F32, tag="ri")
            nc.vector.reciprocal(rowinvC_w, rowprod_w)

            # cE = c^T * e2 chunks (PE transpose quads, e2 folded in drain)
            cE_w = win.tile([P, CHW, P], BF16, tag="cE")
            for x in range(CHW // 4):
                ps_ct = pp_ab.tile([P, 4, P], BF16, tag="ab")
                for k in range(4):
                    j = 4 * x + k
                    nc.tensor.transpose(
                        ps_ct[:, k, :], c_b_w[:, j * P : (j + 1) * P], identity_b
                    )
                for k in range(4):
                    j = 4 * x + k
                    nc.vector.tensor_scalar(
                        cE_w[:, j, :],
                        ps_ct[:, k, :],
                        e2col_w[:, j : j + 1],
                        None,
                        mult,
                    )

            # tmp accumulates across all windows of the batch
            for j in range(CHW):
                nc.tensor.matmul(
                    st["tmp_ps"],
                    lhsT=cE_w[:, j, :],
                    rhs=expS_w[:, j, :],
                    start=(w == 0 and j == 0),
                    stop=(w == NWIN - 1 and j == CHW - 1),
                )

            # pass 1: true exp(s) in [m,n] layout; accum gives colsum partials
            for i in range(MCH):
                ps1 = pp_s.tile([P, WIN], F32, tag="s")
                for h in range(2):
                    nc.tensor.matmul(
                        ps1[:, h * M : (h + 1) * M],
                        lhsT=st["Bq"][:, i * P : (i + 1) * P],
                        rhs=c_b_w[:, h * M : (h + 1) * M],
                        start=True,
                        stop=True,
                    )
                nc.scalar.activation(
                    st["expST"][:, i, lo : lo + WIN],
                    ps1,
                    Exp,
                    bias=st["o1col"][:, i : i + 1],
                    scale=1.0,
                    accum_out=st["colsumU"][:, i, w : w + 1],
                )

            # b-wave of the previous batch rides along
            if b == 1:
                bwave_mm(0, w)

            # a-wave of THIS window (expST for it just emitted)
            awave_mm(t)

            # rowinv -> [1,WIN] row -> broadcast (consumed by the next
            # window's awave_finish; emitted last so it never blocks PE)
            ps_rT = pp_ab.tile([CHW, P], F32, tag="ab")
            nc.tensor.transpose(ps_rT, rowinvC_w, identity)
            rowT_w = win.tile([CHW, P], BF16, tag="rT")
            nc.vector.tensor_copy(rowT_w, ps_rT)
            rowrow_w = win.tile([1, WIN], BF16, tag="rr")
            nc.sync.dma_start(
                out=rowrow_w.rearrange("p (a b) -> p a b", a=CHW), in_=rowT_w
            )
            nc.gpsimd.partition_broadcast(st["rowinvb"][:, lo : lo + WIN], rowrow_w)

        # ---- fill ----
        prep_load(0)
        load_cwin(0)
        load_cwin(1)
        for t in range(T):
            emit_window(t)
        # ---- drain ----
        awave_finish()
        bwave_finish()
        emit_bprep(1)
        for w in range(NWIN):
            bwave_mm(1, w)
            bwave_finish()


_PROGRAM = None


def _build_program(loops=None):
    """Build the per-core Bass program. loops=None -> straight-line (grading
    path); loops=R -> wrap the body in a Tile For_i repetition loop (used
    only for steady-state benchmarking)."""
    nc = bacc.Bacc("TRN2", target_bir_lowering=False, debug=False, num_devices=NCORES)
    q_d = nc.dram_tensor("q", [CB, D, M], F32, kind="ExternalInput")
    c_d = nc.dram_tensor("c", [CB, D, N], F32, kind="ExternalInput")
    w1_d = nc.dram_tensor("w1", [1, D], F32, kind="ExternalInput")
    w2_d = nc.dram_tensor("w2", [1, D], F32, kind="ExternalInput")
    w3_d = nc.dram_tensor("w3", [1, D], F32, kind="ExternalInput")
    out_d = nc.dram_tensor("out", [CB, 4 * D, N], F32, kind="ExternalOutput")
    with tile.TileContext(nc) as tc:
        if loops is None:
            build_body(
                tc, q_d.ap(), c_d.ap(), w1_d.ap(), w2_d.ap(), w3_d.ap(), out_d.ap()
            )
        else:
            with tc.For_i(0, loops, 1):
                build_body(
                    tc,
                    q_d.ap(),
                    c_d.ap(),
                    w1_d.ap(),
                    w2_d.ap(),
                    w3_d.ap(),
                    out_d.ap(),
                )
    nc.compile()
    return nc


def _get_program():
    global _PROGRAM
    if _PROGRAM is None:
        _PROGRAM = _build_program()
    return _PROGRAM


def kernel(q, c, w1, w2, w3, _collect_results=None):
    q = np.ascontiguousarray(q, dtype=np.float32)
    c = np.ascontiguousarray(c, dtype=np.float32)
    w1 = np.ascontiguousarray(w1, dtype=np.float32)
    w2 = np.ascontiguousarray(w2, dtype=np.float32)
    w3 = np.ascontiguousarray(w3, dtype=np.float32)

    nc = _get_program()
    in_maps = [
        {
            "q": q[CB * i : CB * (i + 1)],
            "c": c[CB * i : CB * (i + 1)],
            "w1": w1,
            "w2": w2,
            "w3": w3,
        }
        for i in range(NCORES)
    ]
    from concourse import bass_utils

    res = bass_utils.run_bass_kernel_spmd(nc, in_maps, core_ids=list(range(NCORES)))
    if _collect_results is not None:
        _collect_results.append(res)
    return np.concatenate([r["out"] for r in res.results], axis=0)


# revision 17
# speedup vs baseline: 1.1806x; 1.0702x over previous
"""Trainium2 Bass kernel for ContextQueryAttention (trilinear similarity +
row/col softmax attention).

Full-input contract: kernel(**inputs) takes the complete arrays
  q  [16, 128, 512]   f32
  c  [16, 128, 4096]  f32
  w1 [1, 128] w2 [1, 128] w3 [1, 128] f32
and returns out [16, 512, 4096] f32 = concat([c, a, c*a, c*b], axis=1).

Sharding: data-parallel over batch B=16 across 8 NeuronCores (2 batches per
core), no collectives.

Structure: streaming n-windows of 1024 across both batches (8 global
windows), software-pipelined so PE / ACT / DVE / GPSIMD / DMA overlap:

  per window w (8 n-chunks of 128):
    pass2:  s' = A^T q per chunk -> ACT exp -> expS [n,m] (unscaled; the
            e^{out2[n]} factor cancels in the row softmax)
    rowsum: one DVE reduce per window (off the PE critical path), then
            rowinv = 1/(rowsum*e2) -> PE transpose -> [1,1024] row ->
            gpsimd partition_broadcast
    cE:     PE chunk transposes of c, drained with a per-chunk
            tensor_scalar that folds e2[n] in (cE = c^T * e2)
    tmp:    accumulates cE^T @ expS over all windows (PSUM resident)
    pass1:  s^T = Bq^T c + out1[m] bias -> ACT exp (true exp, FD1024) with
            accum_out -> colsum partials
    a-wave: qT^T @ expST at end of the window (FD1024); its rowinv
            normalization + ca = a*c (gpsimd) land at the head of the next
            window so they never head-of-line block anything
  batch b's b-waves (tmpT^T @ expST, cb = c*b) ride along in batch b+1's
  windows / the tail.

All matmul operands bf16; exp outputs written bf16 by ACT directly; out
block 0 (the c passthrough) is DRAM->DRAM DMA, untouched by compute.
"""

import sys

import numpy as np

try:
    import concourse.bass as bass  # noqa: F401
except Exception:  # pragma: no cover
    sys.path.insert(0, "/opt/trn_rl_repo")
    import concourse.bass as bass  # noqa: F401

import concourse.bacc as bacc
import concourse.mybir as mybir
import concourse.tile as tile
from concourse.masks import make_identity

F32 = mybir.dt.float32
BF16 = mybir.dt.bfloat16

# Problem geometry (hardcoded per contract)
B = 16          # total batches
NCORES = 8
CB = B // NCORES  # batches per core = 2
D = 128         # model dim == partition count
M = 512         # query length
N = 4096        # context length
P = 128
MCH = M // P    # 4 m-chunks of 128
WIN = 1024      # n-window width
NWIN = N // WIN  # 4 windows per batch
CHW = WIN // P   # 8 n-chunks per window
T = CB * NWIN    # 8 global windows


def build_body(tc, q_ap, c_ap, w1_ap, w2_ap, w3_ap, out_ap):
    """Emit the per-core program. q_ap [CB,128,512], c_ap [CB,128,4096],
    w*_ap [1,128], out_ap [CB,512,4096]."""
    from contextlib import ExitStack

    nc = tc.nc
    mult = mybir.AluOpType.mult
    add = mybir.AluOpType.add
    Exp = mybir.ActivationFunctionType.Exp
    AxX = mybir.AxisListType.X

    with ExitStack() as ctx:
        consts = ctx.enter_context(tc.tile_pool(name="consts", bufs=1))
        per = ctx.enter_context(tc.tile_pool(name="per", bufs=2))
        win = ctx.enter_context(tc.tile_pool(name="win", bufs=2))
        outp = ctx.enter_context(tc.tile_pool(name="outp", bufs=2))
        pp_s = ctx.enter_context(tc.tile_pool(name="pp_s", bufs=2, space="PSUM"))
        pp_ab = ctx.enter_context(tc.tile_pool(name="pp_ab", bufs=3, space="PSUM"))

        identity = consts.tile([P, P], F32)
        make_identity(nc, identity)
        identity_b = consts.tile([P, P], BF16)
        nc.vector.tensor_copy(identity_b, identity)
        w1c = consts.tile([P, 1], F32)
        w2c = consts.tile([P, 1], F32)
        w3c = consts.tile([P, 1], F32)
        nc.sync.dma_start(out=w1c, in_=w1_ap.rearrange("o d -> d o"))
        nc.sync.dma_start(out=w2c, in_=w2_ap.rearrange("o d -> d o"))
        nc.sync.dma_start(out=w3c, in_=w3_ap.rearrange("o d -> d o"))
        w1b = consts.tile([P, 1], BF16)
        w2b = consts.tile([P, 1], BF16)
        nc.vector.tensor_copy(w1b, w1c)
        nc.vector.tensor_copy(w2b, w2c)

        state = [dict() for _ in range(CB)]
        pend_a = []
        pend_b = []

        def prep_load(b):
            """Acquire per-batch input tiles and issue loads."""
            st = state[b]
            st["c_t"] = per.tile([P, N], F32, tag="ct", name=f"ct{b}")
            st["q_b"] = per.tile([P, M], BF16, tag="qb", name=f"qb{b}")
            nc.gpsimd.dma_start(out=st["q_b"], in_=q_ap[b])

        def load_cwin(t):
            b, w = divmod(t, NWIN)
            lo = w * WIN
            nc.sync.dma_start(
                out=state[b]["c_t"][:, lo : lo + WIN],
                in_=c_ap[b][:, lo : lo + WIN],
            )

        def prep_compute(b):
            st = state[b]
            q_b = st["q_b"]
            # qT chunks for the a-waves
            qT = per.tile([P, M], BF16, tag="qT", name=f"qT{b}")
            for i in range(MCH):
                ps_q = pp_ab.tile([P, P], BF16, tag="ab")
                nc.tensor.transpose(ps_q, q_b[:, i * P : (i + 1) * P], identity_b)
                nc.vector.tensor_copy(qT[:, i * P : (i + 1) * P], ps_q)
            st["qT"] = qT
            # Bq = q*w3 + w2 (pass-1 stationary)
            Bq = per.tile([P, M], BF16, tag="Bq", name=f"Bq{b}")
            nc.vector.tensor_scalar(Bq, q_b, w3c, w2c, mult, add)
            st["Bq"] = Bq
            # out1[m] column layout (pass-1 bias)
            ps_o1 = pp_ab.tile([P, MCH], F32, tag="ab")
            for i in range(MCH):
                nc.tensor.matmul(
                    ps_o1[:, i : i + 1],
                    lhsT=q_b[:, i * P : (i + 1) * P],
                    rhs=w1b,
                    start=True,
                    stop=True,
                )
            o1col = per.tile([P, MCH], F32, tag="o1", name=f"o1{b}")
            nc.vector.tensor_copy(o1col, ps_o1)
            st["o1col"] = o1col
            # persistent per-batch big tensors + accumulators
            st["expST"] = per.tile([P, MCH, N], BF16, tag="expST", name=f"eST{b}")
            st["rowinvb"] = per.tile([P, N], BF16, tag="rowinvb", name=f"riv{b}")
            st["tmp_ps"] = pp_ab.tile([P, M], F32, tag="tmp", bufs=1, name=f"tmp{b}")
            st["colsumU"] = per.tile([P, MCH, NWIN], F32, tag="csU", name=f"csU{b}")

        def awave_mm(t):
            """a-wave matmuls at the end of window t (expST just written)."""
            b, w = divmod(t, NWIN)
            st = state[b]
            lo = w * WIN
            ps_a = pp_s.tile([P, WIN], F32, tag="s")
            for i in range(MCH):
                for h in range(2):
                    nc.tensor.matmul(
                        ps_a[:, h * M : (h + 1) * M],
                        lhsT=st["qT"][:, i * P : (i + 1) * P],
                        rhs=st["expST"][:, i, lo + h * M : lo + (h + 1) * M],
                        start=(i == 0),
                        stop=(i == MCH - 1),
                    )
            pend_a.append((ps_a, b, lo))

        def awave_finish():
            """Normalize + produce ca + store, at the head of the next window."""
            while pend_a:
                ps_a, b, lo = pend_a.pop(0)
                st = state[b]
                a_t = outp.tile([P, WIN], F32, tag="a")
                nc.vector.tensor_tensor(
                    a_t, ps_a, st["rowinvb"][:, lo : lo + WIN], mult
                )
                nc.sync.dma_start(out=out_ap[b, P : 2 * P, lo : lo + WIN], in_=a_t)
                ca_t = outp.tile([P, WIN], F32, tag="ca")
                nc.gpsimd.tensor_tensor(ca_t, a_t, st["c_t"][:, lo : lo + WIN], mult)
                nc.sync.dma_start(
                    out=out_ap[b, 2 * P : 3 * P, lo : lo + WIN], in_=ca_t
                )

        def emit_bprep(b):
            st = state[b]
            tmpU = per.tile([P, M], BF16, tag="tmpU", name=f"tmpU{b}")
            nc.vector.tensor_copy(tmpU, st["tmp_ps"])
            colsum = per.tile([P, MCH], F32, tag="cs", name=f"cs{b}")
            nc.vector.reduce_sum(colsum, st["colsumU"], axis=AxX)
            colinv = per.tile([P, MCH], F32, tag="colinv", name=f"cinv{b}")
            nc.vector.reciprocal(colinv, colsum)
            tmpT = per.tile([P, M], BF16, tag="tmpT", name=f"tmpT{b}")
            for i in range(MCH):
                ps_tt = pp_ab.tile([P, P], BF16, tag="ab")
                nc.tensor.transpose(ps_tt, tmpU[:, i * P : (i + 1) * P], identity_b)
                nc.vector.tensor_scalar(
                    tmpT[:, i * P : (i + 1) * P],
                    ps_tt,
                    colinv[:, i : i + 1],
                    None,
                    mult,
                )
            st["tmpT"] = tmpT

        def bwave_mm(b, w):
            st = state[b]
            lo = w * WIN
            ps_b = pp_s.tile([P, WIN], F32, tag="s")
            for i in range(MCH):
                for h in range(2):
                    nc.tensor.matmul(
                        ps_b[:, h * M : (h + 1) * M],
                        lhsT=st["tmpT"][:, i * P : (i + 1) * P],
                        rhs=st["expST"][:, i, lo + h * M : lo + (h + 1) * M],
                        start=(i == 0),
                        stop=(i == MCH - 1),
                    )
            pend_b.append((ps_b, b, lo))

        def bwave_finish():
            while pend_b:
                ps_b, b, lo = pend_b.pop(0)
                st = state[b]
                b1_t = outp.tile([P, WIN], F32, tag="b1")
                nc.vector.tensor_tensor(
                    b1_t, ps_b, st["rowinvb"][:, lo : lo + WIN], mult
                )
                cb_t = outp.tile([P, WIN], F32, tag="cb")
                nc.gpsimd.tensor_tensor(cb_t, b1_t, st["c_t"][:, lo : lo + WIN], mult)
                nc.sync.dma_start(
                    out=out_ap[b, 3 * P : 4 * P, lo : lo + WIN], in_=cb_t
                )

        def tmp_mms(t):
            """tmp accumulation for window t's expS (emitted one window later
            so it never waits on ACT)."""
            b, w = divmod(t, NWIN)
            st = state[b]
            cE_w, expS_w = winstash[t]
            for j in range(CHW):
                nc.tensor.matmul(
                    st["tmp_ps"],
                    lhsT=cE_w[:, j, :],
                    rhs=expS_w[:, j, :],
                    start=(w == 0 and j == 0),
                    stop=(w == NWIN - 1 and j == CHW - 1),
                )

        def emit_window(t):
            b, w = divmod(t, NWIN)
            st = state[b]
            lo = w * WIN

            # prefetch c window t+2; batch-1 inputs load just ahead
            if t + 2 < T:
                if (t + 2) % NWIN == 0:
                    prep_load((t + 2) // NWIN)
                load_cwin(t + 2)
            # out block 0 is just c: DRAM->DRAM, no SBUF involved
            nc.sync.dma_start(
                out=out_ap[b, 0:P, lo : lo + WIN], in_=c_ap[b][:, lo : lo + WIN]
            )

            # A = c*w3 + w1 straight from the f32 c window (2x mode); the
            # bf16 copy of c follows for the other consumers
            A_w = win.tile([P, WIN], BF16, tag="A")
            nc.vector.tensor_scalar(A_w, st["c_t"][:, lo : lo + WIN], w3c, w1c, mult, add)
            c_b_w = win.tile([P, WIN], BF16, tag="cbw")
            nc.vector.tensor_copy(c_b_w, st["c_t"][:, lo : lo + WIN])

            if w == 0:
                prep_compute(b)

            # pass 2: expS' = exp(s - out2) per chunk, [n,m] layout
            expS_w = win.tile([P, CHW, M], BF16, tag="expS")
            for jj in range(CHW // 2):
                ps2 = pp_s.tile([P, 2 * M], F32, tag="s")
                for h in range(2):
                    j = 2 * jj + h
                    nc.tensor.matmul(
                        ps2[:, h * M : (h + 1) * M],
                        lhsT=A_w[:, j * P : (j + 1) * P],
                        rhs=st["q_b"],
                        start=True,
                        stop=True,
                    )
                nc.scalar.activation(
                    expS_w[:, 2 * jj : 2 * jj + 2, :], ps2, Exp, bias=0.0, scale=1.0
                )

            # out2 chunk columns -> e2 = exp(out2)
            ps_o2 = pp_ab.tile([P, CHW], F32, tag="ab")
            for j in range(CHW):
                nc.tensor.matmul(
                    ps_o2[:, j : j + 1],
                    lhsT=c_b_w[:, j * P : (j + 1) * P],
                    rhs=w2b,
                    start=True,
                    stop=True,
                )
            e2col_w = win.tile([P, CHW], F32, tag="e2")
            nc.scalar.activation(e2col_w, ps_o2, Exp, bias=0.0, scale=1.0)

            # cE = c^T * e2 chunks (PE transpose quads, e2 folded in drain)
            cE_w = win.tile([P, CHW, P], BF16, tag="cE")
            for x in range(CHW // 2):
                ps_ct = pp_ab.tile([P, 2, P], BF16, tag="ab")
                for k in range(2):
                    j = 2 * x + k
                    nc.tensor.transpose(
                        ps_ct[:, k, :], c_b_w[:, j * P : (j + 1) * P], identity_b
                    )
                for k in range(2):
                    j = 2 * x + k
                    nc.vector.tensor_scalar(
                        cE_w[:, j, :],
                        ps_ct[:, k, :],
                        e2col_w[:, j : j + 1],
                        None,
                        mult,
                    )
            winstash[t] = (cE_w, expS_w)

            # previous window's tmp accumulation (all deps long resolved)
            if t > 0:
                tmp_mms(t - 1)
            if w == 0 and b == 1:
                # batch-0 accumulators now complete (tmp_mms(3) just ran)
                emit_bprep(0)

            # rowsum (off the PE critical path) -> rowinv columns
            rowsumC_w = win.tile([P, CHW], F32, tag="rs")
            nc.vector.reduce_sum(rowsumC_w, expS_w, axis=AxX)
            rowprod_w = win.tile([P, CHW], F32, tag="rp")
            nc.vector.tensor_tensor(rowprod_w, rowsumC_w, e2col_w, mult)
            rowinvC_w = win.tile([P, CHW], F32, tag="ri")
            nc.vector.reciprocal(rowinvC_w, rowprod_w)

            # pass 1: true exp(s) in [m,n] layout; accum gives colsum partials
            for i in range(MCH):
                ps1 = pp_s.tile([P, WIN], F32, tag="s")
                for h in range(2):
                    nc.tensor.matmul(
                        ps1[:, h * M : (h + 1) * M],
                        lhsT=st["Bq"][:, i * P : (i + 1) * P],
                        rhs=c_b_w[:, h * M : (h + 1) * M],
                        start=True,
                        stop=True,
                    )
                nc.scalar.activation(
                    st["expST"][:, i, lo : lo + WIN],
                    ps1,
                    Exp,
                    bias=st["o1col"][:, i : i + 1],
                    scale=1.0,
                    accum_out=st["colsumU"][:, i, w : w + 1],
                )

            # b-wave of the previous batch rides along in this batch's windows
            if b == 1:
                bwave_mm(0, w)
                bwave_finish()

            # rowinv -> [1,WIN] row -> broadcast (consumed by awave_finish
            # at the END of the NEXT window; plenty of slack)
            ps_rT = pp_ab.tile([CHW, P], F32, tag="ab")
            nc.tensor.transpose(ps_rT, rowinvC_w, identity)
            rowT_w = win.tile([CHW, P], BF16, tag="rT")
            nc.vector.tensor_copy(rowT_w, ps_rT)
            rowrow_w = win.tile([1, WIN], BF16, tag="rr")
            nc.sync.dma_start(
                out=rowrow_w.rearrange("p (a b) -> p a b", a=CHW), in_=rowT_w
            )
            nc.gpsimd.partition_broadcast(st["rowinvb"][:, lo : lo + WIN], rowrow_w)

            # a-wave of the previous window (exp1(t-1) finished long ago)
            if t > 0:
                awave_mm(t - 1)
                awave_finish()

        # ---- fill ----
        winstash = {}
        prep_load(0)
        load_cwin(0)
        load_cwin(1)
        for t in range(T):
            emit_window(t)
        # ---- drain ----
        tmp_mms(T - 1)
        awave_mm(T - 1)
        awave_finish()
        emit_bprep(1)
        for w in range(NWIN):
            bwave_mm(1, w)
            bwave_finish()


_PROGRAM = None


def _build_program(loops=None):
    """Build the per-core Bass program. loops=None -> straight-line (grading
    path); loops=R -> wrap the body in a Tile For_i repetition loop (used
    only for steady-state benchmarking)."""
    nc = bacc.Bacc("TRN2", target_bir_lowering=False, debug=False, num_devices=NCORES)
    q_d = nc.dram_tensor("q", [CB, D, M], F32, kind="ExternalInput")
    c_d = nc.dram_tensor("c", [CB, D, N], F32, kind="ExternalInput")
    w1_d = nc.dram_tensor("w1", [1, D], F32, kind="ExternalInput")
    w2_d = nc.dram_tensor("w2", [1, D], F32, kind="ExternalInput")
    w3_d = nc.dram_tensor("w3", [1, D], F32, kind="ExternalInput")
    out_d = nc.dram_tensor("out", [CB, 4 * D, N], F32, kind="ExternalOutput")
    with tile.TileContext(nc) as tc:
        if loops is None:
            build_body(
                tc, q_d.ap(), c_d.ap(), w1_d.ap(), w2_d.ap(), w3_d.ap(), out_d.ap()
            )
        else:
            with tc.For_i(0, loops, 1):
                build_body(
                    tc,
                    q_d.ap(),
                    c_d.ap(),
                    w1_d.ap(),
                    w2_d.ap(),
                    w3_d.ap(),
                    out_d.ap(),
                )
    nc.compile()
    return nc


def _get_program():
    global _PROGRAM
    if _PROGRAM is None:
        _PROGRAM = _build_program()
    return _PROGRAM


def kernel(q, c, w1, w2, w3, _collect_results=None):
    q = np.ascontiguousarray(q, dtype=np.float32)
    c = np.ascontiguousarray(c, dtype=np.float32)
    w1 = np.ascontiguousarray(w1, dtype=np.float32)
    w2 = np.ascontiguousarray(w2, dtype=np.float32)
    w3 = np.ascontiguousarray(w3, dtype=np.float32)

    nc = _get_program()
    in_maps = [
        {
            "q": q[CB * i : CB * (i + 1)],
            "c": c[CB * i : CB * (i + 1)],
            "w1": w1,
            "w2": w2,
            "w3": w3,
        }
        for i in range(NCORES)
    ]
    from concourse import bass_utils

    res = bass_utils.run_bass_kernel_spmd(nc, in_maps, core_ids=list(range(NCORES)))
    if _collect_results is not None:
        _collect_results.append(res)
    return np.concatenate([r["out"] for r in res.results], axis=0)
